# revision 1
# baseline (speedup 1.0000x reference)
"""Trainium2 Bass kernel for BaseAttentionConvolution (7x7 neighborhood attention).

Computation (reference, fp32):
    q = Q @ Wq + bq                     # [B,H,W,64]
    k = K @ Wk + bk                     # [B,H,W,64]
    S[p, (dy,dx)] = q[p] . k[p+(dy,dx)]         (7x7 window, -inf outside image)
    P = softmax(S / 8)
    O[p] = sum_j P[p,j] * V[p+j]        # [B,H,W,128]
    out = relu(O @ Wv + bv)             # [B,H,W,128]

Sharding: B*H = 192 rows split into 8 bands of 24 rows (one per core).
Each core receives its own pre-sliced inputs (SPMD program, per-core data):
  - qt    [128, 2304]   Q rows transposed to channel-major (host transpose)
  - kt    [128, 2880]   K rows + 3-row halo each side, zero-padded, transposed
  - v     [96, 30, 128] V rows + halo, pixel-in-row major (host transpose)
  - kbias [96, 30]      0 for valid k-rows, -30000 for out-of-image halo rows
  - b4    [96, 10*384]  per-k-row window mask (band |k-q|<=3 x valid band rows)
  - weights/biases (replicated)

On-chip algorithm (per core), keys-on-partitions layout:
  qT[64, 2304] = Wq^T @ qt (+bq), kT[64, 2880] = Wk^T @ kt (+bk)   on PE
  for each band of 4 query rows (6 bands):
    for each of the 10 k-rows r the band touches:
      S^T_r[96k, 384q] = kT_r^T . qT_band           (PE)
      E_r = exp(S/8 + kbias_r) * b4_r               (ACT exp + DVE mask-mul)
      outT[128e, 384q] += V_r^T . E_r               (PE, PSUM accumulate)
      den[1, 384q]     += ones^T . E_r              (PE, PSUM accumulate)
    recip = 1/den; transpose per-96 chunks to [96,1] via PE
    out[96q, 128] = relu((outT_chunk^T @ Wv) * recip)   per query row; DMA out

Matmuls run in float32r (fp32 with 11-bit mantissa, 1 cyc/row at N>=256 vs 4
for fp32). The walrus verifier requires every producer feeding an fp32r
matmul to emit fp32r-rounded data, so DRAM inputs on the matmul path are
declared float32r and pre-rounded on host (RNE to 12 dropped bits); on-chip
producers (ACT/DVE) write float32r tiles. Wv is zero-padded to N=256 so the
output projection also streams at 1 cyc/row.
"""

import numpy as np
from contextlib import ExitStack

import concourse.bass as bass
import concourse.bacc as bacc
import concourse.tile as tile
from concourse import mybir
from concourse.bass_utils import run_bass_kernel_spmd

DT = mybir.dt.float32
FR = mybir.dt.float32r
AF = mybir.ActivationFunctionType

# Problem constants (hardcoded per contract)
B, H, W, C, KD, OD = 2, 96, 96, 128, 64, 128
KS, PAD = 7, 3
NCORES = 8
ROWS = (B * H) // NCORES        # 24 query rows per core
KROWS = ROWS + 2 * PAD          # 30 k/v rows per core (with halo)
NQ = ROWS * W                   # 2304 query pixels per core
NK = KROWS * W                  # 2880 key pixels per core
BAND = 4                        # query rows per band
NBANDS = ROWS // BAND           # 6
BN = BAND * W                   # 384 band query columns
NKR = BAND + 2 * PAD            # 10 k-rows per band
NEG = -30000.0                  # effectively -inf after exp
SCALE = 1.0 / np.sqrt(KD)       # 1/8
WVN = 2 * OD                    # Wv padded free dim (f32r wants N>=256)

# matmul dtype knob: "f32r" (fast, 11-bit mantissa) or "f32" (exact, 4 cyc/row)
MM_DTYPE = "f32r"


def build_nc(mm_dtype=MM_DTYPE, with_bv=False, reps=1):
    MDT = FR if mm_dtype == "f32r" else DT
    nc = bacc.Bacc(None, target_bir_lowering=False)
    qt = nc.dram_tensor("qt", [C, NQ], MDT, kind="ExternalInput")
    kt = nc.dram_tensor("kt", [C, NK], MDT, kind="ExternalInput")
    v = nc.dram_tensor("v", [W, KROWS, C], MDT, kind="ExternalInput")
    wq = nc.dram_tensor("wq", [C, KD], MDT, kind="ExternalInput")
    wk = nc.dram_tensor("wk", [C, KD], MDT, kind="ExternalInput")
    wv = nc.dram_tensor("wv", [C, WVN], MDT, kind="ExternalInput")
    bq = nc.dram_tensor("bq", [KD, 1], DT, kind="ExternalInput")
    bk = nc.dram_tensor("bk", [KD, 1], DT, kind="ExternalInput")
    bv = nc.dram_tensor("bv", [1, WVN], MDT, kind="ExternalInput")
    kbias = nc.dram_tensor("kbias", [W, KROWS], DT, kind="ExternalInput")
    ones_in = nc.dram_tensor("ones", [W, 1], MDT, kind="ExternalInput")
    b4 = nc.dram_tensor("b4", [W, NKR * BN], DT, kind="ExternalInput")
    out = nc.dram_tensor("out", [ROWS, W, OD], DT, kind="ExternalOutput")

    with tile.TileContext(nc) as tc, ExitStack() as ctx:
        consts = ctx.enter_context(tc.tile_pool(name="consts", bufs=1))
        slabs = ctx.enter_context(tc.tile_pool(name="slabs", bufs=1))
        e_pool = ctx.enter_context(tc.tile_pool(name="e_pool", bufs=3))
        o_pool = ctx.enter_context(tc.tile_pool(name="o_pool", bufs=2))
        r_pool = ctx.enter_context(tc.tile_pool(name="r_pool", bufs=2))
        rs_pool = ctx.enter_context(tc.tile_pool(name="rs_pool", bufs=8))
        outs = ctx.enter_context(tc.tile_pool(name="outs", bufs=3))
        ps_a = ctx.enter_context(tc.tile_pool(name="ps_a", bufs=3, space="PSUM"))
        ps_b = ctx.enter_context(tc.tile_pool(name="ps_b", bufs=2, space="PSUM"))
        ps_c = ctx.enter_context(tc.tile_pool(name="ps_c", bufs=2, space="PSUM"))

        for _rep in range(reps):
            # ---- constants ----
            wq_s = consts.tile([C, KD], MDT, tag="cw")
            nc.sync.dma_start(out=wq_s[:], in_=wq[:])
            wk_s = consts.tile([C, KD], MDT, tag="cw2")
            nc.sync.dma_start(out=wk_s[:], in_=wk[:])
            wv_s = consts.tile([C, WVN], MDT, tag="cw3")
            nc.sync.dma_start(out=wv_s[:], in_=wv[:])
            bq_s = consts.tile([KD, 1], DT, tag="cb")
            nc.sync.dma_start(out=bq_s[:], in_=bq[:])
            bk_s = consts.tile([KD, 1], DT, tag="cb2")
            nc.sync.dma_start(out=bk_s[:], in_=bk[:])
            kbias_s = consts.tile([W, KROWS], DT, tag="ckb")
            nc.sync.dma_start(out=kbias_s[:], in_=kbias[:])
            b4_s = consts.tile([W, NKR * BN], DT, tag="cb4")
            nc.sync.dma_start(out=b4_s[:], in_=b4[:])
            ones96 = consts.tile([W, 1], MDT, tag="cones")
            nc.sync.dma_start(out=ones96[:], in_=ones_in[:])
            ones1 = consts.tile([1, 1], DT, tag="cone1")
            nc.vector.memset(ones1[:], 1.0)
            if with_bv:
                bv_s = consts.tile([1, WVN], MDT, tag="cbv")
                nc.sync.dma_start(out=bv_s[:], in_=bv[:])

            # ---- slabs ----
            qt_s = slabs.tile([C, NQ], MDT, tag="sqt")
            nc.sync.dma_start(out=qt_s[:], in_=qt[:])
            kt_s = slabs.tile([C, NK], MDT, tag="skt")
            nc.sync.dma_start(out=kt_s[:], in_=kt[:])
            v_s = slabs.tile([W, KROWS, C], MDT, tag="sv")
            nc.sync.dma_start(out=v_s[:], in_=v[:])

            # ---- projections: qT = Wq^T @ qt + bq ; kT = Wk^T @ kt + bk ----
            qT_s = slabs.tile([KD, NQ], MDT, tag="sqT")
            kT_s = slabs.tile([KD, NK], MDT, tag="skT")
            for dst, src, wmat, bvec, n in (
                (qT_s, qt_s, wq_s, bq_s, NQ),
                (kT_s, kt_s, wk_s, bk_s, NK),
            ):
                for j0 in range(0, n, 512):
                    j1 = min(j0 + 512, n)
                    ps = ps_a.tile([KD, 512], DT, tag="w")
                    nc.tensor.matmul(
                        out=ps[:, : j1 - j0],
                        lhsT=wmat[:],
                        rhs=src[:, j0:j1],
                        start=True,
                        stop=True,
                    )
                    nc.scalar.activation(
                        dst[:, j0:j1], ps[:, : j1 - j0], AF.Identity, bias=bvec[:], scale=1.0
                    )

            # ---- bands ----
            for band in range(NBANDS):
                h0 = band * BAND
                jq = slice(h0 * W, (h0 + BAND) * W)
                outT = ps_b.tile([OD, BN], DT, tag="outT")
                den = ps_c.tile([1, BN], DT, tag="den")
                for i in range(NKR):
                    r = h0 + i  # k-slab row index (slab row 0 = query row -3)
                    S = ps_a.tile([W, BN], DT, tag="w")
                    nc.tensor.matmul(
                        out=S[:],
                        lhsT=kT_s[:, r * W : (r + 1) * W],
                        rhs=qT_s[:, jq],
                        start=True,
                        stop=True,
                    )
                    E = e_pool.tile([W, BN], MDT, tag="E")
                    nc.scalar.activation(
                        E[:], S[:], AF.Exp, bias=kbias_s[:, r : r + 1], scale=SCALE
                    )
                    nc.vector.tensor_mul(E[:], E[:], b4_s[:, i * BN : (i + 1) * BN])
                    nc.tensor.matmul(
                        out=outT[:],
                        lhsT=v_s[:, r, :],
                        rhs=E[:],
                        start=(i == 0),
                        stop=(i == NKR - 1),
                    )
                    nc.tensor.matmul(
                        out=den[:],
                        lhsT=ones96[:],
                        rhs=E[:],
                        start=(i == 0),
                        stop=(i == NKR - 1),
                    )

                # finalize band
                recip = r_pool.tile([1, BN], DT, tag="recip")
                nc.vector.reciprocal(recip[:], den[:])
                oT = o_pool.tile([OD, BN], MDT, tag="oT")
                nc.vector.tensor_copy(oT[:], outT[:])
                if with_bv:
                    den_sb = r_pool.tile([1, BN], MDT, tag="densb")
                    nc.vector.tensor_copy(den_sb[:], den[:])
                for c in range(BAND):
                    cs = slice(c * W, (c + 1) * W)
                    rT = ps_a.tile([W, 1], DT, tag="w")
                    nc.tensor.transpose(rT[:], recip[:, cs], ones1[:])
                    rS = rs_pool.tile([W, 1], DT, tag="rS")
                    nc.vector.tensor_copy(rS[:], rT[:])
                    op = ps_a.tile([W, WVN], DT, tag="w")
                    nc.tensor.matmul(
                        out=op[:],
                        lhsT=oT[:, cs],
                        rhs=wv_s[:],
                        start=True,
                        stop=not with_bv,
                    )
                    if with_bv:
                        nc.tensor.matmul(
                            out=op[:],
                            lhsT=den_sb[:, cs],
                            rhs=bv_s[:],
                            start=False,
                            stop=True,
                        )
                    ost = outs.tile([W, OD], DT, tag="ost")
                    nc.scalar.activation(ost[:], op[:, :OD], AF.Relu, bias=0.0, scale=rS[:])
                    nc.sync.dma_start(out=out[h0 + c], in_=ost[:])

    nc.compile()
    return nc


def round_f32r(x):
    """Round fp32 -> fp32r bit pattern (1s8e11m, low 12 bits zero, RNE)."""
    b = np.ascontiguousarray(x, np.float32).view(np.uint32)
    tie = (b >> 12) & 1
    b = (b + 0x7FF + tie) & np.uint32(0xFFFFF000)
    return b.view(np.float32)


def make_in_maps(Q, K, V, Wq, bq, Wk, bk, Wv, bv, mm_dtype=None):
    if mm_dtype is None:
        mm_dtype = MM_DTYPE
    rnd = round_f32r if mm_dtype == "f32r" else lambda x: np.ascontiguousarray(x, np.float32)

    Q = np.asarray(Q, np.float32)
    K = np.asarray(K, np.float32)
    V = np.asarray(V, np.float32)
    Wqr = rnd(np.asarray(Wq, np.float32))
    Wkr = rnd(np.asarray(Wk, np.float32))
    wvp = np.zeros((C, WVN), np.float32)
    wvp[:, :OD] = np.asarray(Wv, np.float32)
    wvp = rnd(wvp)
    bqv = np.ascontiguousarray(np.asarray(bq, np.float32).reshape(KD, 1))
    bkv = np.ascontiguousarray(np.asarray(bk, np.float32).reshape(KD, 1))
    bvp = np.zeros((1, WVN), np.float32)
    bvp[0, :OD] = np.asarray(bv, np.float32)
    bvp = rnd(bvp)

    # per-k-row mask [96, 10, 384]: within-row band (|k-q|<=PAD) for the valid
    # band query-rows of each k-row i (i-2*PAD <= c <= i), zero elsewhere
    idx = np.arange(W)
    b4 = (np.abs(idx[:, None] - idx[None, :]) <= PAD).astype(np.float32)
    b4i = np.zeros((W, NKR, BAND, W), np.float32)
    for i in range(NKR):
        for c in range(BAND):
            if i - 2 * PAD <= c <= i:
                b4i[:, i, c, :] = b4
    b4rep = np.ascontiguousarray(b4i.reshape(W, NKR * BAND * W))

    in_maps = []
    for core in range(NCORES):
        b = core // (H // ROWS)
        h_start = (core % (H // ROWS)) * ROWS

        qs = Q[b, h_start : h_start + ROWS]  # [24,96,128]
        qtc = rnd(np.ascontiguousarray(qs.reshape(NQ, C).T))  # [128,2304]

        kpad = np.zeros((KROWS, W, C), np.float32)
        vpad = np.zeros((KROWS, W, C), np.float32)
        kb = np.full((KROWS,), NEG, np.float32)
        for j in range(KROWS):
            g = h_start - PAD + j
            if 0 <= g < H:
                kpad[j] = K[b, g]
                vpad[j] = V[b, g]
                kb[j] = 0.0
        ktc = rnd(np.ascontiguousarray(kpad.reshape(NK, C).T))  # [128,2880]
        vtc = rnd(np.ascontiguousarray(vpad.transpose(1, 0, 2)))  # [96,30,128]
        kbias = np.ascontiguousarray(np.broadcast_to(kb[None, :], (W, KROWS)))

        in_maps.append(
            {
                "qt": qtc,
                "kt": ktc,
                "v": vtc,
                "wq": Wqr,
                "wk": Wkr,
                "wv": wvp,
                "bq": bqv,
                "bk": bkv,
                "bv": bvp,
                "kbias": kbias,
                "ones": np.ones((W, 1), np.float32),
                "b4": b4rep,
            }
        )
    return in_maps


def gather(results):
    full = np.empty((B, H, W, OD), np.float32)
    for core in range(NCORES):
        b = core // (H // ROWS)
        h_start = (core % (H // ROWS)) * ROWS
        full[b, h_start : h_start + ROWS] = results[core]["out"]
    return full


_NC_CACHE = {}


def get_nc(mm_dtype=MM_DTYPE, with_bv=False, reps=1):
    key = (mm_dtype, with_bv, reps)
    if key not in _NC_CACHE:
        _NC_CACHE[key] = build_nc(mm_dtype=mm_dtype, with_bv=with_bv, reps=reps)
    return _NC_CACHE[key]


def kernel(Q, K, V, Wq, bq, Wk, bk, Wv, bv):
    with_bv = bool(np.any(np.asarray(bv)))
    nc = get_nc(MM_DTYPE, with_bv)
    in_maps = make_in_maps(Q, K, V, Wq, bq, Wk, bk, Wv, bv, mm_dtype=MM_DTYPE)
    res = run_bass_kernel_spmd(nc, in_maps, list(range(NCORES)))
    return gather(res.results)



# revision 3
# speedup vs baseline: 1.4639x; 1.4639x over previous
"""Trainium2 Bass kernel for BaseAttentionConvolution (7x7 neighborhood attention).

Computation (reference, fp32):
    q = Q @ Wq + bq                     # [B,H,W,64]
    k = K @ Wk + bk                     # [B,H,W,64]
    S[p, (dy,dx)] = q[p] . k[p+(dy,dx)]         (7x7 window, -inf outside image)
    P = softmax(S / 8)
    O[p] = sum_j P[p,j] * V[p+j]        # [B,H,W,128]
    out = relu(O @ Wv + bv)             # [B,H,W,128]

Sharding: B*H = 192 rows split into 8 bands of 24 rows (one per core).
Each core receives its own pre-sliced inputs (SPMD program, per-core data):
  - qt     [128, 2304]  Q rows transposed to channel-major (host transpose), bf16
  - kt     [128, 2912]  K rows + 3-row halo, zero-padded (+32 pad cols), bf16
  - v      [96, 30, 128] V rows + halo, pixel-in-row major, bf16
  - b4     [96, 384]    banded |k-x|<=3 mask tiled 4x along free dim, bf16
  - excess [96, 24]     den correction: #out-of-image window rows x band width
  - weights/biases (replicated), bf16 weights / fp32 biases

On-chip algorithm (per core), keys-on-partitions, everything bf16 on the
matmul path with fp32 PSUM accumulation:
  qT[64, 2304] = Wq^T @ qt (+bq), kT[64, 2912] = Wk^T @ kt (+bk)   on PE
  bands of 4 query rows, processed in interleaved PAIRS to hide the
  PE->ACT->DVE->PE chain latency; per (band, k-row r = band*4+i):
    S[128k, sub] = kT_r(128 wide, FWL) ^T . qT_band_sub     (PE)
    E[96, sub]   = exp(S/8)                                  (ACT, bf16 out)
    E *= b4                                                  (DVE, banded dx mask)
    outT[128e, 384] += V_r^T . E    (PE, PSUM acc; full width at i=0/9,
    den[1, 384]     += ones^T . E    E zero-padded by gpsimd memset there)
  sub = the 96-col query blocks actually in range (|qrow-krow|<=3); out-of-
  image k-rows contribute exp(0)*b4 to den, removed via the host-computed
  `excess` correction (V is zero there so outT is unaffected).
  finalize: den -> SBUF, 4x PE transpose to [96,1], den -= excess, DVE
  reciprocal; out[q,:] = relu((outT_chunk^T @ Wv) * recip) per query row.
"""

import numpy as np
from contextlib import ExitStack

import ml_dtypes

import concourse.bass as bass
import concourse.bacc as bacc
import concourse.tile as tile
from concourse import mybir
from concourse.bass_utils import run_bass_kernel_spmd

DT = mybir.dt.float32
BF = mybir.dt.bfloat16
AF = mybir.ActivationFunctionType
ALU = mybir.AluOpType

# Problem constants (hardcoded per contract)
B, H, W, C, KD, OD = 2, 96, 96, 128, 64, 128
KS, PAD = 7, 3
NCORES = 8
ROWS = (B * H) // NCORES        # 24 query rows per core
KROWS = ROWS + 2 * PAD          # 30 k/v rows per core (with halo)
NQ = ROWS * W                   # 2304 query pixels per core
NK = KROWS * W                  # 2880 key pixels per core
NKP = NK + 32                   # kt padded so lhsT slices can be 128 wide (FWL)
BAND = 4                        # query rows per band
NBANDS = ROWS // BAND           # 6
BN = BAND * W                   # 384 band query columns
NKR = BAND + 2 * PAD            # 10 k-rows per band
SCALE = 1.0 / np.sqrt(KD)       # 1/8

MM_DTYPE = "bf16"


def build_nc(mm_dtype=MM_DTYPE, with_bv=False, reps=1):
    nc = bacc.Bacc(None, target_bir_lowering=False)
    qt = nc.dram_tensor("qt", [C, NQ], BF, kind="ExternalInput")
    kt = nc.dram_tensor("kt", [C, NKP], BF, kind="ExternalInput")
    v = nc.dram_tensor("v", [W, KROWS, C], BF, kind="ExternalInput")
    wq = nc.dram_tensor("wq", [C, KD], BF, kind="ExternalInput")
    wk = nc.dram_tensor("wk", [C, KD], BF, kind="ExternalInput")
    wv = nc.dram_tensor("wv", [C, OD], BF, kind="ExternalInput")
    bq = nc.dram_tensor("bq", [KD, 1], DT, kind="ExternalInput")
    bk = nc.dram_tensor("bk", [KD, 1], DT, kind="ExternalInput")
    bv = nc.dram_tensor("bv", [1, OD], BF, kind="ExternalInput")
    ones_in = nc.dram_tensor("ones", [W, 1], BF, kind="ExternalInput")
    b4 = nc.dram_tensor("b4", [W, BN], BF, kind="ExternalInput")
    excess = nc.dram_tensor("excess", [W, ROWS], DT, kind="ExternalInput")
    exflat = nc.dram_tensor("exflat", [1, NQ], DT, kind="ExternalInput")
    out = nc.dram_tensor("out", [ROWS, W, OD], DT, kind="ExternalOutput")

    with tile.TileContext(nc) as tc, ExitStack() as ctx:
        consts = ctx.enter_context(tc.tile_pool(name="consts", bufs=1))
        slabs = ctx.enter_context(tc.tile_pool(name="slabs", bufs=1))
        e_pool = ctx.enter_context(tc.tile_pool(name="e_pool", bufs=5))
        o_pool = ctx.enter_context(tc.tile_pool(name="o_pool", bufs=2))
        dn_pool = ctx.enter_context(tc.tile_pool(name="dn_pool", bufs=2))
        outs = ctx.enter_context(tc.tile_pool(name="outs", bufs=2))
        # PSUM: ps_a holds S / projection / output-proj tiles (1 bank each),
        # ps_o the outT accumulators, ps_d the den accumulators (+4 transpose
        # columns). 4 + 2 + 2 = 8 banks.
        ps_a = ctx.enter_context(tc.tile_pool(name="ps_a", bufs=4, space="PSUM"))
        ps_o = ctx.enter_context(tc.tile_pool(name="ps_o", bufs=2, space="PSUM"))
        ps_d = ctx.enter_context(tc.tile_pool(name="ps_d", bufs=2, space="PSUM"))

        for _rep in range(reps):
            # ---- constants ----
            wq_s = consts.tile([C, KD], BF, tag="cwq")
            nc.sync.dma_start(out=wq_s[:], in_=wq[:])
            wk_s = consts.tile([C, KD], BF, tag="cwk")
            nc.sync.dma_start(out=wk_s[:], in_=wk[:])
            wv_s = consts.tile([C, OD], BF, tag="cwv")
            nc.sync.dma_start(out=wv_s[:], in_=wv[:])
            bq_s = consts.tile([KD, 1], DT, tag="cbq")
            nc.sync.dma_start(out=bq_s[:], in_=bq[:])
            bk_s = consts.tile([KD, 1], DT, tag="cbk")
            nc.sync.dma_start(out=bk_s[:], in_=bk[:])
            b4_s = consts.tile([W, BN], BF, tag="cb4")
            nc.sync.dma_start(out=b4_s[:], in_=b4[:])
            ones96 = consts.tile([W, 1], BF, tag="cones")
            nc.sync.dma_start(out=ones96[:], in_=ones_in[:])
            excess_s = consts.tile([W, ROWS], DT, tag="cex")
            nc.sync.dma_start(out=excess_s[:], in_=excess[:])
            ones1 = consts.tile([1, 1], DT, tag="cone1")
            nc.vector.memset(ones1[:], 1.0)
            if with_bv:
                bv_s = consts.tile([1, OD], BF, tag="cbv")
                nc.sync.dma_start(out=bv_s[:], in_=bv[:])
                exflat_s = consts.tile([1, NQ], DT, tag="cexf")
                nc.sync.dma_start(out=exflat_s[:], in_=exflat[:])

            # ---- slabs ----
            qt_s = slabs.tile([C, NQ], BF, tag="sqt")
            nc.sync.dma_start(out=qt_s[:], in_=qt[:])
            kt_s = slabs.tile([C, NKP], BF, tag="skt")
            nc.sync.dma_start(out=kt_s[:], in_=kt[:])
            v_s = slabs.tile([W, KROWS, C], BF, tag="sv")
            nc.sync.dma_start(out=v_s[:], in_=v[:])

            # ---- projections: qT = Wq^T @ qt + bq ; kT = Wk^T @ kt + bk ----
            qT_s = slabs.tile([KD, NQ], BF, tag="sqT")
            kT_s = slabs.tile([KD, NKP], BF, tag="skT")
            for dst, src, wmat, bvec, n, on_act in (
                (qT_s, qt_s, wq_s, bq_s, NQ, False),
                (kT_s, kt_s, wk_s, bk_s, NKP, True),
            ):
                for j0 in range(0, n, 512):
                    j1 = min(j0 + 512, n)
                    ps = ps_a.tile([C, 512], DT, tag="w")
                    nc.tensor.matmul(
                        out=ps[:KD, : j1 - j0],
                        lhsT=wmat[:],
                        rhs=src[:, j0:j1],
                        start=True,
                        stop=True,
                    )
                    if on_act:
                        nc.scalar.activation(
                            dst[:, j0:j1], ps[:KD, : j1 - j0], AF.Identity,
                            bias=bvec[:], scale=1.0,
                        )
                    else:
                        nc.vector.tensor_scalar_add(
                            dst[:, j0:j1], ps[:KD, : j1 - j0], bvec[:]
                        )

            # ---- bands, processed as interleaved pairs ----
            state = {}        # (band) -> dict with psum tiles / E tiles
            pending_finalize = []

            def front(band, i):
                """S matmul + exp + mask for k-row i of `band`."""
                st = state[band]
                h0 = band * BAND
                r = h0 + i
                c_lo, c_hi = max(0, i - 6), min(3, i)
                lo, hi = c_lo * W, (c_hi + 1) * W
                wdt = hi - lo
                S = ps_a.tile([C, 512], DT, tag="w")
                nc.tensor.matmul(
                    out=S[:, lo:hi],
                    lhsT=kt_proj[:, r * W : r * W + C],
                    rhs=qT_s[:, h0 * W + lo : h0 * W + hi],
                    start=True,
                    stop=True,
                )
                E = e_pool.tile([W, BN], BF, tag="E")
                if i == 0:
                    nc.gpsimd.memset(E[:, hi:BN], 0.0)
                elif i == NKR - 1:
                    nc.gpsimd.memset(E[:, 0:lo], 0.0)
                nc.scalar.activation(
                    E[:, lo:hi], S[:W, lo:hi], AF.Exp, bias=0.0, scale=SCALE
                )
                nc.vector.tensor_mul(E[:, lo:hi], E[:, lo:hi], b4_s[:, lo:hi])
                st["E"][i] = E

            def back(band, i):
                """outT/den accumulation for k-row i of `band`."""
                st = state[band]
                h0 = band * BAND
                r = h0 + i
                c_lo, c_hi = max(0, i - 6), min(3, i)
                full = i == 0 or i == NKR - 1
                lo, hi = (0, BN) if full else (c_lo * W, (c_hi + 1) * W)
                E = st["E"][i]
                nc.tensor.matmul(
                    out=st["outT"][:, lo:hi],
                    lhsT=v_s[:, r, :],
                    rhs=E[:, lo:hi],
                    start=(i == 0),
                    stop=(i == NKR - 1),
                )
                nc.tensor.matmul(
                    out=st["den"][0:1, lo:hi],
                    lhsT=ones96[:],
                    rhs=E[:, lo:hi],
                    start=(i == 0),
                    stop=(i == NKR - 1),
                )

            def finalize_copies(band):
                """PSUM -> SBUF copies that free outT/den (emit right after
                the last back() of the band)."""
                st = state[band]
                denS = dn_pool.tile([1, BN], DT, tag="denS")
                nc.vector.tensor_copy(denS[:], st["den"][0:1, 0:BN])
                oT = o_pool.tile([OD, BN], BF, tag="oT")
                nc.vector.tensor_copy(oT[:], st["outT"][:])
                st["denS"], st["oT"] = denS, oT

            def finalize_rest(band):
                """Transpose den, reciprocal, Wv projection, relu, DMA out."""
                st = state[band]
                h0 = band * BAND
                denS, oT = st["denS"], st["oT"]
                rt = st["den"]  # reuse den psum tile's spare columns
                for c in range(BAND):
                    nc.tensor.transpose(
                        rt[0:W, BN + c : BN + c + 1],
                        denS[:, c * W : (c + 1) * W],
                        ones1[:],
                    )
                rT4 = dn_pool.tile([W, BAND], DT, tag="rT4")
                nc.vector.tensor_sub(
                    rT4[:], rt[0:W, BN : BN + BAND],
                    excess_s[:, h0 : h0 + BAND],
                )
                recipS = dn_pool.tile([W, BAND], DT, tag="recipS")
                nc.vector.reciprocal(recipS[:], rT4[:])
                if with_bv:
                    denc = dn_pool.tile([1, BN], BF, tag="denc")
                    nc.vector.tensor_sub(
                        denc[:], denS[:], exflat_s[:, h0 * W : h0 * W + BN]
                    )
                ost = outs.tile([W, BAND, OD], DT, tag="ost")
                op = ps_a.tile([C, 512], DT, tag="w")
                for c in range(BAND):
                    cs = slice(c * W, (c + 1) * W)
                    nc.tensor.matmul(
                        out=op[:W, c * OD : (c + 1) * OD],
                        lhsT=oT[:, cs],
                        rhs=wv_s[:],
                        start=True,
                        stop=not with_bv,
                    )
                    if with_bv:
                        nc.tensor.matmul(
                            out=op[:W, c * OD : (c + 1) * OD],
                            lhsT=denc[:, cs],
                            rhs=bv_s[:],
                            start=False,
                            stop=True,
                        )
                    if c % 2 == 0:
                        nc.scalar.activation(
                            ost[:, c, :], op[:W, c * OD : (c + 1) * OD],
                            AF.Relu, bias=0.0, scale=recipS[:, c : c + 1],
                        )
                    else:
                        nc.vector.tensor_scalar(
                            ost[:, c, :], op[:W, c * OD : (c + 1) * OD],
                            recipS[:, c : c + 1], 0.0,
                            op0=ALU.mult, op1=ALU.max,
                        )
                    nc.sync.dma_start(out=out[h0 + c], in_=ost[:, c, :])

            kt_proj = kT_s
            DEPTH = 2  # back() runs DEPTH slots behind front()
            for pair in range(NBANDS // 2):
                bands = (2 * pair, 2 * pair + 1)
                for bd in bands:
                    state[bd] = {
                        "E": {},
                        "outT": ps_o.tile([OD, BN], DT, tag="outT", name="outT"),
                        "den": ps_d.tile([W, BN + BAND], DT, tag="den", name="den"),
                    }
                slots = [(bd, i) for i in range(NKR) for bd in bands]
                for s, (bd, i) in enumerate(slots):
                    front(bd, i)
                    if s == 1 and pending_finalize:
                        for pbd in pending_finalize:
                            finalize_rest(pbd)
                        pending_finalize.clear()
                    if s >= DEPTH:
                        back(*slots[s - DEPTH])
                for s in range(len(slots) - DEPTH, len(slots)):
                    back(*slots[s])
                for bd in bands:
                    finalize_copies(bd)
                    pending_finalize.append(bd)
            for pbd in pending_finalize:
                finalize_rest(pbd)

    nc.compile()
    return nc


def make_in_maps(Q, K, V, Wq, bq, Wk, bk, Wv, bv, mm_dtype=None):
    BFN = ml_dtypes.bfloat16

    Q = np.asarray(Q, np.float32)
    K = np.asarray(K, np.float32)
    V = np.asarray(V, np.float32)
    wqb = np.ascontiguousarray(np.asarray(Wq, np.float32)).astype(BFN)
    wkb = np.ascontiguousarray(np.asarray(Wk, np.float32)).astype(BFN)
    wvb = np.ascontiguousarray(np.asarray(Wv, np.float32)).astype(BFN)
    bqv = np.ascontiguousarray(np.asarray(bq, np.float32).reshape(KD, 1))
    bkv = np.ascontiguousarray(np.asarray(bk, np.float32).reshape(KD, 1))
    bvv = np.ascontiguousarray(np.asarray(bv, np.float32).reshape(1, OD)).astype(BFN)

    # banded |k-x|<=3 mask, tiled BAND times along the free dim
    idx = np.arange(W)
    Bm = (np.abs(idx[:, None] - idx[None, :]) <= PAD).astype(np.float32)
    b4rep = np.ascontiguousarray(np.tile(Bm, (1, BAND))).astype(BFN)
    bw = Bm.sum(axis=0)  # [x] band width per column

    in_maps = []
    for core in range(NCORES):
        b = core // (H // ROWS)
        h_start = (core % (H // ROWS)) * ROWS

        qs = Q[b, h_start : h_start + ROWS]  # [24,96,128]
        qtc = np.ascontiguousarray(qs.reshape(NQ, C).T).astype(BFN)

        kpad = np.zeros((KROWS, W, C), np.float32)
        vpad = np.zeros((KROWS, W, C), np.float32)
        inv = np.zeros((ROWS,), np.float32)
        for j in range(KROWS):
            g = h_start - PAD + j
            if 0 <= g < H:
                kpad[j] = K[b, g]
                vpad[j] = V[b, g]
        for c in range(ROWS):
            inv[c] = sum(1 for dy in range(-PAD, PAD + 1)
                         if not (0 <= h_start + c + dy < H))
        ktc = np.zeros((C, NKP), np.float32)
        ktc[:, :NK] = kpad.reshape(NK, C).T
        ktc = np.ascontiguousarray(ktc).astype(BFN)
        vtc = np.ascontiguousarray(vpad.transpose(1, 0, 2)).astype(BFN)
        # den correction [x, qrow] and flat [1, NQ] (bv path)
        exc = np.ascontiguousarray(bw[:, None] * inv[None, :]).astype(np.float32)
        excf = np.ascontiguousarray(
            (inv[:, None] * bw[None, :]).reshape(1, NQ)
        ).astype(np.float32)

        in_maps.append(
            {
                "qt": qtc,
                "kt": ktc,
                "v": vtc,
                "wq": wqb,
                "wk": wkb,
                "wv": wvb,
                "bq": bqv,
                "bk": bkv,
                "bv": bvv,
                "ones": np.ones((W, 1), BFN),
                "b4": b4rep,
                "excess": exc,
                "exflat": excf,
            }
        )
    return in_maps


def gather(results):
    full = np.empty((B, H, W, OD), np.float32)
    for core in range(NCORES):
        b = core // (H // ROWS)
        h_start = (core % (H // ROWS)) * ROWS
        full[b, h_start : h_start + ROWS] = results[core]["out"]
    return full


_NC_CACHE = {}


def get_nc(mm_dtype=MM_DTYPE, with_bv=False, reps=1):
    key = (mm_dtype, with_bv, reps)
    if key not in _NC_CACHE:
        _NC_CACHE[key] = build_nc(mm_dtype=mm_dtype, with_bv=with_bv, reps=reps)
    return _NC_CACHE[key]


def kernel(Q, K, V, Wq, bq, Wk, bk, Wv, bv):
    with_bv = bool(np.any(np.asarray(bv)))
    nc = get_nc(MM_DTYPE, with_bv)
    in_maps = make_in_maps(Q, K, V, Wq, bq, Wk, bk, Wv, bv, mm_dtype=MM_DTYPE)
    res = run_bass_kernel_spmd(nc, in_maps, list(range(NCORES)))
    return gather(res.results)
